# revision 1
# baseline (speedup 1.0000x reference)
"""Cross-attention with LoRA (Q and V adapters) on 8 TRN2 NeuronCores.

Sharding: core = (b, g) where b = batch index (2), g = head group (4 groups
of 4 heads).  Data parallel over batch, tensor parallel over heads for the
QKV projections; the output projection is column-sharded so each core
produces a partial (1024, 2048) output that the host sums per batch.

All device tensors are pre-transposed on the host so the kernel needs no
on-chip transposes:
  xt   = x[b].T               (1024, 2048)   [embed, seq]
  ctxt = context[b].T         (1024, 2048)   [embed, ctx]
  wqT  = (Wq[g]/8).T          (1024, 256)    1/sqrt(hd) folded in
  bqT  = (Bq[g]/(128*8)).T    (128, 256)     LoRA 1/r and 1/sqrt(hd) folded
  wkT  = Wk[g].T              (1024, 256)
  wvT  = Wv[g].T              (1024, 256)
  aqT/avT = Aq.T/Av.T         (1024, 128)    replicated
  bvT  = (Bv[g]/128).T        (128, 256)
  woT  = Wo[:, g].T           (256, 1024)
Output out_t = (x-partial of out).T per core; host computes
  out[b] = sum_g out_t[(b,g)].T
"""

import numpy as np

import concourse.bass as bass
import concourse.tile as tile
from concourse import bacc, mybir
from concourse.bass import ts
from concourse.bass_utils import run_bass_kernel_spmd

F32 = mybir.dt.float32
BF16 = mybir.dt.bfloat16
EXP = mybir.ActivationFunctionType.Exp

P = 128          # partitions
D = 1024         # embed dim
KO = D // P      # embed chunks (8)
HG = 4           # heads per core
HD = 64          # head dim
GD = HG * HD     # group dim (256)
R = 128          # LoRA rank
NMM = 512        # matmul moving-dim chunk
AQ = 512         # activation streaming quarter (phase-1 seq chunk)
SQB = 1024       # phase-2 query block


def build_nc(S=2048, C=2048):
    """Build + compile the per-core Bass program (identical on all cores)."""
    nc = bacc.Bacc("TRN2", target_bir_lowering=False, debug=False)

    xt = nc.dram_tensor("xt", [D, S], BF16, kind="ExternalInput").ap()
    ctxt = nc.dram_tensor("ctxt", [D, C], BF16, kind="ExternalInput").ap()
    wqT = nc.dram_tensor("wqT", [D, GD], BF16, kind="ExternalInput").ap()
    aqT = nc.dram_tensor("aqT", [D, R], BF16, kind="ExternalInput").ap()
    bqT = nc.dram_tensor("bqT", [R, GD], BF16, kind="ExternalInput").ap()
    wkT = nc.dram_tensor("wkT", [D, GD], BF16, kind="ExternalInput").ap()
    wvT = nc.dram_tensor("wvT", [D, GD], BF16, kind="ExternalInput").ap()
    avT = nc.dram_tensor("avT", [D, R], BF16, kind="ExternalInput").ap()
    bvT = nc.dram_tensor("bvT", [R, GD], BF16, kind="ExternalInput").ap()
    woT = nc.dram_tensor("woT", [GD, D], BF16, kind="ExternalInput").ap()
    out_t = nc.dram_tensor("out_t", [D, S], F32, kind="ExternalOutput").ap()

    with tile.TileContext(nc) as tc:
        _build(tc, xt, ctxt, wqT, aqT, bqT, wkT, wvT, avT, bvT, woT, out_t, S, C)
    nc.compile()
    return nc


def _build(tc, xt, ctxt, wqT, aqT, bqT, wkT, wvT, avT, bvT, woT, out_t, S, C):
    nc = tc.nc
    CK = C // P      # context seq chunks (16)
    sqb = min(SQB, S)  # phase-2 query block

    xt_r = xt.rearrange("(ko p) s -> p ko s", p=P)
    ctxt_r = ctxt.rearrange("(ko p) s -> p ko s", p=P)
    out_r = out_t.rearrange("(ko p) s -> ko p s", p=P)

    with (
        tc.tile_pool(name="w", bufs=1) as wpool,
        tc.tile_pool(name="wbig", bufs=2) as wbig,
        tc.tile_pool(name="acts", bufs=2) as actsp,
        tc.tile_pool(name="kqv", bufs=1) as kqv,
        tc.tile_pool(name="lora", bufs=1) as lorap,
        tc.tile_pool(name="pt", bufs=4) as ptp,
        tc.tile_pool(name="small", bufs=2) as smallp,
        tc.tile_pool(name="outsb", bufs=2) as outp,
    ):
        # ---- small weights (resident) ----
        aq_sb = wpool.tile([P, KO, R], BF16, tag="aq")
        nc.sync.dma_start(aq_sb[:], aqT.rearrange("(ko p) r -> p ko r", p=P))
        av_sb = wpool.tile([P, KO, R], BF16, tag="av")
        nc.sync.dma_start(av_sb[:], avT.rearrange("(ko p) r -> p ko r", p=P))
        bq_sb = wpool.tile([R, GD], BF16, tag="bq")
        nc.sync.dma_start(bq_sb[:], bqT)
        bv_sb = wpool.tile([R, GD], BF16, tag="bv")
        nc.sync.dma_start(bv_sb[:], bvT)

        # ---- big weights cycle through 2 slots: wk, wv, wq, wo ----
        wk_sb = wbig.tile([P, KO, GD], BF16, tag="wbig")
        nc.sync.dma_start(wk_sb[:], wkT.rearrange("(ko p) m -> p ko m", p=P))
        wv_sb = wbig.tile([P, KO, GD], BF16, tag="wbig")
        nc.sync.dma_start(wv_sb[:], wvT.rearrange("(ko p) m -> p ko m", p=P))

        # ---- persistent activations ----
        # kt_z / vaug_z are zero-padded so every phase-2 matmul drives the
        # FULL 128x128 PE array (half-array matmuls keep the HAM clock gate
        # cold at 1.2 GHz -- measured 427 ns/MM instead of 213 ns).
        # kt_z[:, h]: rows (h%2)*64..+64 hold K_h^T, other 64 rows are zero.
        # vaug_z[:, sk, h]: cols 0..63 = V_h, col 64 = ones, cols 65..127 = 0.
        kt_z = kqv.tile([P, HG, C], BF16, tag="kt")       # K^T  [hd, ctx]
        qt_sb = kqv.tile([P, 2, S], BF16, tag="qt")       # Q^T  [hd, seq]
        vaug_z = kqv.tile([P, CK, HG, P], BF16, tag="vaug")
        att_sb = kqv.tile([P, 2, S], BF16, tag="att")     # attn out^T (normalized)
        tv_sb = lorap.tile([R, C], BF16, tag="tv")
        tq_sb = lorap.tile([R, S], BF16, tag="tq")

        nc.vector.memset(kt_z[:], 0.0)
        nc.vector.memset(vaug_z[:], 0.0)
        nc.vector.memset(vaug_z[:, :, :, HD], 1.0)

        # ================= phase 1a: context -> Kt, V =================
        with (
            tc.tile_pool(name="psum1", bufs=4, space="PSUM") as psum1,
            tc.tile_pool(name="psumv", bufs=2, space="PSUM") as psumv,
        ):
            for q in range(C // AQ):
                sl = slice(q * AQ, (q + 1) * AQ)
                ctx_sb = actsp.tile([P, KO, AQ], BF16, tag="acts")
                nc.sync.dma_start(ctx_sb[:], ctxt_r[:, :, sl])

                # tv = Av @ ctx^T  -> [R, ctx-quarter]
                tvp = psum1.tile([P, NMM], F32, tag="proj")
                for k in range(KO):
                    nc.tensor.matmul(
                        tvp[:], (av_sb[:, k, :]), (ctx_sb[:, k, :]),
                        start=(k == 0), stop=(k == KO - 1),
                    )
                nc.vector.tensor_copy(tv_sb[:, sl], tvp[:])

                # Kt quarter (rows 0:64 -> head 2m, rows 64:128 -> head 2m+1)
                for m in range(2):
                    kp = psum1.tile([P, NMM], F32, tag="proj")
                    for k in range(KO):
                        nc.tensor.matmul(
                            kp[:], (wk_sb[:, k, ts(m, P)]), (ctx_sb[:, k, :]),
                            start=(k == 0), stop=(k == KO - 1),
                        )
                    nc.vector.tensor_copy(kt_z[0:HD, 2 * m, sl], kp[0:HD, :])
                    nc.vector.tensor_copy(kt_z[HD:P, 2 * m + 1, sl], kp[HD:P, :])

                # V quarter (normal layout, head-interleaved with ones col)
                for mloc in range(AQ // P):
                    vp = psumv.tile([P, GD], F32, tag="vproj")
                    for k in range(KO):
                        nc.tensor.matmul(
                            vp[:], (ctx_sb[:, k, ts(mloc, P)]), (wv_sb[:, k, :]),
                            start=(k == 0), stop=False,
                        )
                    nc.tensor.matmul(
                        vp[:], (tv_sb[:, q * AQ + mloc * P:q * AQ + (mloc + 1) * P]),
                        (bv_sb[:]), start=False, stop=True,
                    )
                    mg = q * (AQ // P) + mloc
                    nc.vector.tensor_copy(
                        vaug_z[:, mg, :, 0:HD],
                        vp[:].rearrange("p (h d) -> p h d", h=HG),
                    )

            # ================= phase 1b: x -> Qt =================
            wq_sb = wbig.tile([P, KO, GD], BF16, tag="wbig")
            nc.sync.dma_start(wq_sb[:], wqT.rearrange("(ko p) m -> p ko m", p=P))

            for q in range(S // AQ):
                sl = slice(q * AQ, (q + 1) * AQ)
                x_sb = actsp.tile([P, KO, AQ], BF16, tag="acts")
                nc.sync.dma_start(x_sb[:], xt_r[:, :, sl])

                tqp = psum1.tile([P, NMM], F32, tag="proj")
                for k in range(KO):
                    nc.tensor.matmul(
                        tqp[:], (aq_sb[:, k, :]), (x_sb[:, k, :]),
                        start=(k == 0), stop=(k == KO - 1),
                    )
                nc.vector.tensor_copy(tq_sb[:, sl], tqp[:])

                for m in range(2):
                    qp = psum1.tile([P, NMM], F32, tag="proj")
                    for k in range(KO):
                        nc.tensor.matmul(
                            qp[:], (wq_sb[:, k, ts(m, P)]), (x_sb[:, k, :]),
                            start=(k == 0), stop=False,
                        )
                    nc.tensor.matmul(
                        qp[:], (bq_sb[:, ts(m, P)]), (tq_sb[:, sl]),
                        start=False, stop=True,
                    )
                    nc.vector.tensor_copy(qt_sb[:, m, sl], qp[:])

        # ================= phase 2: attention =================
        wo_sb = wbig.tile([P, 2, D], BF16, tag="wbig")
        nc.sync.dma_start(wo_sb[:], woT.rearrange("(j p) d -> p j d", p=P))

        with (
            tc.tile_pool(name="st", bufs=2, space="PSUM") as stp,
            tc.tile_pool(name="ot", bufs=2, space="PSUM") as otp,
        ):
            for qb in range(S // sqb):
                for h in range(HG):
                    hp = (h % 2) * HD
                    hc = h // 2
                    ot = otp.tile([P, sqb], F32, tag="ot")

                    def attn_v(sk, pt):
                        for n in range(sqb // NMM):
                            nc.tensor.matmul(
                                ot[:, ts(n, NMM)],
                                (vaug_z[:, sk, h, :]),
                                (pt[:, ts(n, NMM)]),
                                start=(sk == 0), stop=(sk == CK - 1),
                            )

                    # software-pipelined: attnV for iteration sk-1 is emitted
                    # after scores/exp of iteration sk, so the PE stream never
                    # head-of-line blocks on the current iteration's ACT.
                    prev = None
                    for sk in range(CK):
                        st = stp.tile([P, sqb], F32, tag="st")
                        for n in range(sqb // NMM):
                            nc.tensor.matmul(
                                st[:, ts(n, NMM)],
                                (kt_z[:, h, ts(sk, P)]),
                                (qt_sb[:, hc,
                                       qb * sqb + n * NMM:qb * sqb + (n + 1) * NMM]),
                                start=True, stop=True,
                            )
                        pt = ptp.tile([P, sqb], BF16, tag="pt")
                        nc.scalar.activation(pt[:], st[:], EXP)
                        if prev is not None:
                            attn_v(*prev)
                        prev = (sk, pt)
                    attn_v(*prev)
                    # normalize: rows 0..63 are O^T, row 64 is the exp rowsum
                    rr = smallp.tile([1, sqb], F32, tag="rr")
                    nc.vector.tensor_copy(rr[:], ot[HD:HD + 1, :])
                    rf = smallp.tile([1, sqb], F32, tag="rf")
                    nc.vector.reciprocal_approx_fast(rf[:], rr[:])
                    rb = smallp.tile([HD, sqb], F32, tag="rb")
                    nc.gpsimd.partition_broadcast(rb[:], rf[:])
                    nc.vector.tensor_mul(
                        att_sb[hp:hp + HD, hc,
                               qb * sqb:(qb + 1) * sqb],
                        ot[0:HD, :], rb[:],
                    )

                # ---- out-projection for this query block (PSUM via st tag) ----
                for e in range(KO):
                    osb = outp.tile([P, sqb], F32, tag="osb")
                    for n in range(sqb // NMM):
                        ng = qb * (sqb // NMM) + n
                        op = stp.tile([P, NMM], F32, tag="st")
                        for j in range(2):
                            nc.tensor.matmul(
                                op[:], (wo_sb[:, j, ts(e, P)]),
                                (att_sb[:, j, ts(ng, NMM)]),
                                start=(j == 0), stop=(j == 1),
                            )
                        nc.vector.tensor_copy(osb[:, ts(n, NMM)], op[:])
                    nc.sync.dma_start(out_r[e][:, qb * sqb:(qb + 1) * sqb], osb[:])


# ---------------------------------------------------------------------------
# Host side
# ---------------------------------------------------------------------------

_NC_CACHE = {}


def _get_nc(S=2048, C=2048):
    key = (S, C)
    if key not in _NC_CACHE:
        _NC_CACHE[key] = build_nc(S, C)
    return _NC_CACHE[key]


def shard_inputs(x, context, Wq, Aq, Bq, Wk, Wv, Av, Bv, Wo):
    """Build the 8 per-core input maps (host-side shard + transpose + scale +
    bf16 cast)."""
    import ml_dtypes

    bf16 = ml_dtypes.bfloat16
    f = lambda a: np.ascontiguousarray(np.asarray(a, dtype=np.float32))
    c = lambda a: np.ascontiguousarray(a).astype(bf16)
    x, context = f(x), f(context)
    Wq, Aq, Bq, Wk, Wv, Av, Bv, Wo = map(f, (Wq, Aq, Bq, Wk, Wv, Av, Bv, Wo))
    sd = 8.0  # sqrt(head_dim)
    lr = 128.0  # LoRA rank (scale = 1/r)
    aqT = c(Aq.T)
    avT = c(Av.T)
    in_maps = []
    for core in range(8):
        b, g = core // 4, core % 4
        sl = slice(g * GD, (g + 1) * GD)
        in_maps.append({
            "xt": c(x[b].T),
            "ctxt": c(context[b].T),
            "wqT": c(Wq[sl].T / sd),
            "aqT": aqT,
            "bqT": c(Bq[sl].T / (lr * sd)),
            "wkT": c(Wk[sl].T),
            "wvT": c(Wv[sl].T),
            "avT": avT,
            "bvT": c(Bv[sl].T / lr),
            "woT": c(Wo[:, sl].T),
        })
    return in_maps


def unshard_output(results, B=2, S=2048):
    out = np.zeros((B, S, D), np.float32)
    for core in range(8):
        b = core // 4
        out[b] += results[core]["out_t"].T
    return out


def kernel(x, context, Wq, Aq, Bq, Wk, Wv, Av, Bv, Wo, _trace=False):
    nc = _get_nc()
    in_maps = shard_inputs(x, context, Wq, Aq, Bq, Wk, Wv, Av, Bv, Wo)
    res = run_bass_kernel_spmd(nc, in_maps, core_ids=list(range(8)), trace=_trace)
    out = unshard_output(res.results)
    if _trace:
        kernel.last_result = res
    return out



# revision 18
# speedup vs baseline: 1.0120x; 1.0120x over previous
"""Cross-attention with LoRA (Q and V adapters) on 8 TRN2 NeuronCores.

Sharding: core = (b, g): b = batch index (2), g = head group (4 groups of 4
heads).  Data parallel over batch, tensor parallel over heads; the output
projection is column-sharded so each core produces a partial (1024, 2048)
output that the host sums per batch.

Fused single-pass pipeline.  The softmax EXP on the Scalar engine (~136us
total) is the phase-2 critical resource, so all projection work (context
quarters 2-3, Q for the second query block, out-projection of the first) is
injected as filler quanta between attention iterations to keep the PE busy
while ACT chews exp.  Per-head K^T/Q^T are duplicated into both partition
halves so the two query halves of a scores chunk run concurrently on array
tiles T0/T8 (row tiling, 2x scores throughput).

Host-side layouts (partition-major so DMA lines are large; all dram tensors
declared 2-D):
  xt/ctxt  [128, 8*2048]    embed e = p*8+k
  wqT/wkT/wvT [128, 8*256]  rows match the e = p*8+k mapping
  aqT/avT  [128, 8*128]
  bqT/bvT  [128, 256]       contraction = LoRA rank on partitions
  woT      [128, 2*1024]    row (p, j) = att layout dim j*128+p
  out_t    [1024, 2048]     f32, row e*128+p; host out[b] += out_t.T
Scale folding as baseline: 1/sqrt(hd) in wq/bq, 1/r in bq/bv.
"""

import numpy as np

import concourse.bass as bass
import concourse.tile as tile
from concourse import bacc, mybir
from concourse.bass import ts
from concourse.bass_utils import run_bass_kernel_spmd

F32 = mybir.dt.float32
BF16 = mybir.dt.bfloat16
EXP = mybir.ActivationFunctionType.Exp

# scores matmul mode: "pair" = chunk's two query halves on array tiles
# T0/T8 concurrently (2x scores throughput); "t0" = both halves on T0
# (64-contraction, no packing); "full" = 128-contraction over the duplicated
# halves (head counted twice, host folds an extra 1/2 into wk).
SCORES_MODE = "pair"

P = 128          # partitions
D = 1024         # embed dim
KO = D // P      # embed chunks (8)
HG = 4           # heads per core
HD = 64          # head dim
GD = HG * HD     # group dim (256)
R = 128          # LoRA rank
SQ = 1024        # query block (exp instruction free size)
QTR = 512        # ctx quarter


def build_nc(S=2048, C=2048):
    nc = bacc.Bacc("TRN2", target_bir_lowering=False, debug=False)

    xt = nc.dram_tensor("xt", [P, KO * S], BF16, kind="ExternalInput").ap()
    ctxt = nc.dram_tensor("ctxt", [P, KO * C], BF16, kind="ExternalInput").ap()
    wqT = nc.dram_tensor("wqT", [P, KO * GD], BF16, kind="ExternalInput").ap()
    aqT = nc.dram_tensor("aqT", [P, KO * R], BF16, kind="ExternalInput").ap()
    bqT = nc.dram_tensor("bqT", [R, GD], BF16, kind="ExternalInput").ap()
    wkT = nc.dram_tensor("wkT", [P, KO * GD], BF16, kind="ExternalInput").ap()
    wvT = nc.dram_tensor("wvT", [P, KO * GD], BF16, kind="ExternalInput").ap()
    avT = nc.dram_tensor("avT", [P, KO * R], BF16, kind="ExternalInput").ap()
    bvT = nc.dram_tensor("bvT", [R, GD], BF16, kind="ExternalInput").ap()
    woT = nc.dram_tensor("woT", [P, 2 * D], BF16, kind="ExternalInput").ap()
    out_t = nc.dram_tensor("out_t", [D, S], F32, kind="ExternalOutput").ap()

    xt = xt.rearrange("p (ko s) -> p ko s", ko=KO)
    ctxt = ctxt.rearrange("p (ko s) -> p ko s", ko=KO)
    wqT = wqT.rearrange("p (ko g) -> p ko g", ko=KO)
    aqT = aqT.rearrange("p (ko r) -> p ko r", ko=KO)
    wkT = wkT.rearrange("p (ko g) -> p ko g", ko=KO)
    wvT = wvT.rearrange("p (ko g) -> p ko g", ko=KO)
    avT = avT.rearrange("p (ko r) -> p ko r", ko=KO)
    woT = woT.rearrange("p (j d) -> p j d", j=2)
    out_t = out_t.rearrange("(ko p) s -> ko p s", p=P)
    with tile.TileContext(nc) as tc:
        _build(tc, xt, ctxt, wqT, aqT, bqT, wkT, wvT, avT, bvT, woT, out_t, S, C)
    nc.compile()
    return nc


def _build(tc, xt, ctxt, wqT, aqT, bqT, wkT, wvT, avT, bvT, woT, out_t, S, C):
    nc = tc.nc
    CK = C // P       # 16 ctx chunks
    NCQ = C // QTR    # 4 ctx quarters

    with (
        tc.tile_pool(name="w", bufs=1) as wpool,
        tc.tile_pool(name="acts", bufs=1) as actsp,
        tc.tile_pool(name="kqv", bufs=1) as kqv,
        tc.tile_pool(name="lora", bufs=1) as lorap,
        tc.tile_pool(name="pt", bufs=6) as ptp,
        tc.tile_pool(name="small", bufs=2) as smallp,
        tc.tile_pool(name="outsb", bufs=2) as outp,
        tc.tile_pool(name="work", bufs=3, space="PSUM") as work,
        tc.tile_pool(name="otp", bufs=1, space="PSUM") as otp,
    ):
        # ---- persistent SBUF tensors ----
        ctx_sb = actsp.tile([P, KO, C], BF16, tag="ctx")
        x_sb = actsp.tile([P, KO, S], BF16, tag="x")
        # per-head K^T / Q^T, duplicated into both partition halves so the
        # two query halves of a chunk can run on array tiles T0 and T8
        kt_sb = kqv.tile([P, HG, C], BF16, tag="kt")
        qt_sb = kqv.tile([P, HG, S], BF16, tag="qt")
        vaug_sb = kqv.tile([P, CK, HG, P], BF16, tag="vaug")
        att_sb = kqv.tile([P, 2, S], BF16, tag="att")   # normalized attn out^T
        tv_sb = lorap.tile([R, C], BF16, tag="tv")
        tq_sb = lorap.tile([R, S], BF16, tag="tq")

        aq_sb = wpool.tile([P, KO, R], BF16, tag="aq")
        av_sb = wpool.tile([P, KO, R], BF16, tag="av")
        bq_sb = wpool.tile([R, GD], BF16, tag="bq")
        bv_sb = wpool.tile([R, GD], BF16, tag="bv")
        wk_sb = wpool.tile([P, KO, GD], BF16, tag="wk")
        wv_sb = wpool.tile([P, KO, GD], BF16, tag="wv")
        wq_sb = wpool.tile([P, KO, GD], BF16, tag="wq")
        wo_sb = wpool.tile([P, 2, D], BF16, tag="wo")

        # ---- input DMAs, ordered by first use ----
        nc.sync.dma_start(ctx_sb[:, :, 0:QTR], ctxt[:, :, 0:QTR])
        nc.sync.dma_start(av_sb[:], avT)
        nc.sync.dma_start(wk_sb[:], wkT)
        nc.sync.dma_start(wv_sb[:], wvT)
        nc.sync.dma_start(bv_sb[:], bvT)
        nc.sync.dma_start(ctx_sb[:, :, QTR:2 * QTR], ctxt[:, :, QTR:2 * QTR])
        nc.sync.dma_start(x_sb[:, :, 0:SQ], xt[:, :, 0:SQ])
        nc.sync.dma_start(aq_sb[:], aqT)
        nc.sync.dma_start(wq_sb[:], wqT)
        nc.sync.dma_start(bq_sb[:], bqT)
        nc.sync.dma_start(ctx_sb[:, :, 2 * QTR:3 * QTR], ctxt[:, :, 2 * QTR:3 * QTR])
        nc.sync.dma_start(ctx_sb[:, :, 3 * QTR:4 * QTR], ctxt[:, :, 3 * QTR:4 * QTR])
        nc.sync.dma_start(x_sb[:, :, SQ:2 * SQ], xt[:, :, SQ:2 * SQ])
        nc.sync.dma_start(wo_sb[:], woT)

        # ones column for the attn-V rowsum; zero the pad columns so the
        # (unread) ot rows 65..127 accumulate zeros, not junk
        nc.vector.memset(vaug_sb[:, :, :, HD:P], 0.0)
        nc.vector.memset(vaug_sb[:, :, :, HD], 1.0)

        # ---- projection emitters (closures so they can be used as fillers) ----
        def quarter_closures(q):
            """Project ctx quarter q -> tv, Kt, Vaug.  4 filler quanta."""
            sl = ts(q, QTR)

            def tv():
                wt = work.tile([P, SQ], F32, tag="work")
                for k in range(KO):
                    nc.tensor.matmul(wt[:, 0:QTR], av_sb[:, k, :], ctx_sb[:, k, sl],
                                     start=(k == 0), stop=(k == KO - 1))
                nc.vector.tensor_copy(tv_sb[:, sl], wt[:, 0:QTR])

            def kproj(m):
                def f():
                    wt = work.tile([P, SQ], F32, tag="work")
                    for k in range(KO):
                        nc.tensor.matmul(wt[:, 0:QTR], wk_sb[:, k, ts(m, P)],
                                         ctx_sb[:, k, sl],
                                         start=(k == 0), stop=(k == KO - 1))
                    nc.vector.tensor_copy(kt_sb[0:HD, 2 * m, sl], wt[0:HD, 0:QTR])
                    nc.vector.tensor_copy(kt_sb[HD:P, 2 * m + 1, sl], wt[HD:P, 0:QTR])
                    nc.sync.dma_start(kt_sb[HD:P, 2 * m, sl], kt_sb[0:HD, 2 * m, sl])
                    nc.sync.dma_start(kt_sb[0:HD, 2 * m + 1, sl],
                                      kt_sb[HD:P, 2 * m + 1, sl])
                return f

            def vproj():
                wt = work.tile([P, SQ], F32, tag="work")
                vpq = wt.rearrange("p (m g) -> p m g", m=4)
                for mloc in range(4):
                    cpos = q * QTR + mloc * P
                    for k in range(KO):
                        nc.tensor.matmul(vpq[:, mloc, :], ctx_sb[:, k, cpos:cpos + P],
                                         wv_sb[:, k, :], start=(k == 0), stop=False)
                    nc.tensor.matmul(vpq[:, mloc, :], tv_sb[:, cpos:cpos + P],
                                     bv_sb[:], start=False, stop=True)
                nc.vector.tensor_copy(
                    vaug_sb[:, q * 4:(q + 1) * 4, :, 0:HD],
                    vpq.rearrange("p m (h d) -> p m h d", h=HG))

            return [tv, kproj(0), kproj(1), vproj]

        def q_closures(qb):
            """Project query block qb -> tq, Qt.  3 filler quanta."""
            sl = ts(qb, SQ)

            def tq():
                wt = work.tile([P, SQ], F32, tag="work")
                for n in range(2):
                    nsl = slice(qb * SQ + n * 512, qb * SQ + (n + 1) * 512)
                    for k in range(KO):
                        nc.tensor.matmul(wt[:, ts(n, 512)], aq_sb[:, k, :],
                                         x_sb[:, k, nsl],
                                         start=(k == 0), stop=(k == KO - 1))
                nc.vector.tensor_copy(tq_sb[:, sl], wt[:])

            def qproj(m):
                def f():
                    wt = work.tile([P, SQ], F32, tag="work")
                    for n in range(2):
                        nsl = slice(qb * SQ + n * 512, qb * SQ + (n + 1) * 512)
                        for k in range(KO):
                            nc.tensor.matmul(wt[:, ts(n, 512)],
                                             wq_sb[:, k, ts(m, P)], x_sb[:, k, nsl],
                                             start=(k == 0), stop=False)
                        nc.tensor.matmul(wt[:, ts(n, 512)], bq_sb[:, ts(m, P)],
                                         tq_sb[:, nsl], start=False, stop=True)
                    nc.vector.tensor_copy(qt_sb[0:HD, 2 * m, sl], wt[0:HD, :])
                    nc.vector.tensor_copy(qt_sb[HD:P, 2 * m + 1, sl], wt[HD:P, :])
                    nc.sync.dma_start(qt_sb[HD:P, 2 * m, sl], qt_sb[0:HD, 2 * m, sl])
                    nc.sync.dma_start(qt_sb[0:HD, 2 * m + 1, sl],
                                      qt_sb[HD:P, 2 * m + 1, sl])
                return f

            return [tq, qproj(0), qproj(1)]

        def outproj_closures(qb):
            """Out-projection of query block qb.  8 filler quanta (one per
            128-wide output chunk)."""
            sl = ts(qb, SQ)

            def op(e):
                def f():
                    wt = work.tile([P, SQ], F32, tag="work")
                    for n in range(2):
                        nsl = slice(qb * SQ + n * 512, qb * SQ + (n + 1) * 512)
                        for j in range(2):
                            nc.tensor.matmul(wt[:, ts(n, 512)],
                                             wo_sb[:, j, ts(e, P)], att_sb[:, j, nsl],
                                             start=(j == 0), stop=(j == 1))
                    ob = outp.tile([P, SQ], F32, tag="osb")
                    nc.vector.tensor_copy(ob[:], wt[:])
                    nc.sync.dma_start(out_t[e][:, sl], ob[:])
                return f

            return [op(e) for e in range(KO)]

        # ---- attention for one (query block, head) with filler injection ----
        def attention(qb, h, fillers, dens):
            hc, hp = h // 2, (h % 2) * HD
            qsl = ts(qb, SQ)
            ot = otp.tile([P, SQ], F32, tag="ot")
            pts = {}

            def attnv(i):
                pt = pts.pop(i)
                for n in range(2):
                    nc.tensor.matmul(ot[:, ts(n, 512)], vaug_sb[:, i, h, :],
                                     pt[:, ts(n, 512)],
                                     start=(i == 0), stop=(i == CK - 1))

            for i in range(CK):
                st = work.tile([P, SQ], F32, tag="work")
                nsl = lambda n: slice(qb * SQ + n * 512, qb * SQ + (n + 1) * 512)
                if SCORES_MODE == "full":
                    for n in range(2):
                        nc.tensor.matmul(st[:, ts(n, 512)], kt_sb[:, h, ts(i, P)],
                                         qt_sb[:, h, nsl(n)], start=True, stop=True)
                elif SCORES_MODE == "t0":
                    for n in range(2):
                        nc.tensor.matmul(st[:, ts(n, 512)], kt_sb[0:HD, h, ts(i, P)],
                                         qt_sb[0:HD, h, nsl(n)], start=True, stop=True)
                else:  # "pair": query half 0 on T0, half 1 on T8, concurrent
                    nc.tensor.matmul(st[:, 0:512], kt_sb[0:HD, h, ts(i, P)],
                                     qt_sb[0:HD, h, nsl(0)], start=True, stop=True)
                    nc.tensor.matmul(st[:, 512:1024], kt_sb[HD:P, h, ts(i, P)],
                                     qt_sb[HD:P, h, nsl(1)], start=True, stop=True)
                pt = ptp.tile([P, SQ], BF16, tag="pt")
                nc.scalar.activation(pt[:], st[:], EXP)
                pts[i] = pt
                if i >= 2:
                    attnv(i - 2)
                for _ in range(dens):
                    if fillers:
                        fillers.pop(0)()
            attnv(CK - 2)
            attnv(CK - 1)

            # normalize: rows 0..63 are O^T, row 64 is the exp rowsum.
            # Exact baseline chain: hop the rowsum through a partition-0
            # [1, N] tile before the reciprocal; multiply reads ot from PSUM.
            rr = smallp.tile([1, SQ], F32, tag="rr")
            nc.vector.tensor_copy(rr[:], ot[HD:HD + 1, :])
            rf = smallp.tile([1, SQ], F32, tag="rf")
            nc.vector.reciprocal_approx_fast(rf[:], rr[:])
            rb = smallp.tile([HD, SQ], F32, tag="rb")
            nc.gpsimd.partition_broadcast(rb[:], rf[:])
            nc.vector.tensor_mul(att_sb[hp:hp + HD, hc, qsl], ot[0:HD, :], rb[:])

        # ================= emission =================
        # prologue: ctx quarters 0-1 and Q(qb0) inline
        for q in range(2):
            for c in quarter_closures(q):
                c()
        for c in q_closures(0):
            c()

        # qb0: fillers = ctx quarters 2-3 (needed by scores chunk >= 8 of h0,
        # emitted 2/iter so they land in the PE queue early), then Q(qb1).
        f0 = []
        for q in range(2, NCQ):
            f0 += quarter_closures(q)
        f0 += q_closures(1)
        attention(0, 0, f0, 2)
        attention(0, 1, f0, 1)
        attention(0, 2, f0, 1)
        attention(0, 3, f0, 1)
        assert not f0

        # qb1: fillers = out-projection of qb0
        f1 = outproj_closures(0)
        attention(1, 0, f1, 1)
        attention(1, 1, f1, 1)
        attention(1, 2, f1, 1)
        attention(1, 3, f1, 1)
        assert not f1

        # tail: out-projection of qb1
        for c in outproj_closures(1):
            c()


# ---------------------------------------------------------------------------
# Host side
# ---------------------------------------------------------------------------

_NC_CACHE = {}


def _get_nc(S=2048, C=2048):
    key = (S, C)
    if key not in _NC_CACHE:
        _NC_CACHE[key] = build_nc(S, C)
    return _NC_CACHE[key]


def shard_inputs(x, context, Wq, Aq, Bq, Wk, Wv, Av, Bv, Wo):
    """Build the 8 per-core input maps (host-side shard + transpose + scale +
    bf16 cast, partition-major 2-D layouts)."""
    import ml_dtypes

    bf16 = ml_dtypes.bfloat16
    f = lambda a: np.ascontiguousarray(np.asarray(a, dtype=np.float32))
    c = lambda a: np.ascontiguousarray(a).astype(bf16)
    x, context = f(x), f(context)
    Wq, Aq, Bq, Wk, Wv, Av, Bv, Wo = map(f, (Wq, Aq, Bq, Wk, Wv, Av, Bv, Wo))
    sd = 8.0   # sqrt(head_dim)
    lr = 128.0  # LoRA rank (scale = 1/r)
    aqT = c(Aq.T.reshape(P, KO * R))
    avT = c(Av.T.reshape(P, KO * R))
    in_maps = []
    for core in range(8):
        b, g = core // 4, core % 4
        sl = slice(g * GD, (g + 1) * GD)
        in_maps.append({
            "xt": c(x[b].T.reshape(P, KO * 2048)),
            "ctxt": c(context[b].T.reshape(P, KO * 2048)),
            "wqT": c((Wq[sl].T / sd).reshape(P, KO * GD)),
            "aqT": aqT,
            "bqT": c(Bq[sl].T / (lr * sd)),
            "wkT": c((Wk[sl] / (2.0 if SCORES_MODE == "full" else 1.0))
                     .T.reshape(P, KO * GD)),
            "wvT": c(Wv[sl].T.reshape(P, KO * GD)),
            "avT": avT,
            "bvT": c(Bv[sl].T / lr),
            "woT": c(np.ascontiguousarray(
                Wo[:, sl].T.reshape(2, P, D).swapaxes(0, 1)).reshape(P, 2 * D)),
        })
    return in_maps


def unshard_output(results, B=2, S=2048):
    out = np.zeros((B, S, D), np.float32)
    for core in range(8):
        b = core // 4
        out[b] += results[core]["out_t"].reshape(D, S).astype(np.float32).T
    return out


def kernel(x, context, Wq, Aq, Bq, Wk, Wv, Av, Bv, Wo, _trace=False):
    nc = _get_nc()
    in_maps = shard_inputs(x, context, Wq, Aq, Bq, Wk, Wv, Av, Bv, Wo)
    res = run_bass_kernel_spmd(nc, in_maps, core_ids=list(range(8)), trace=_trace)
    out = unshard_output(res.results)
    if _trace:
        kernel.last_result = res
    return out


# revision 19
# speedup vs baseline: 1.1104x; 1.0972x over previous
"""Cross-attention with LoRA (Q and V adapters) on 8 TRN2 NeuronCores.

Sharding: core = (b, g): b = batch index (2), g = head group (4 groups of 4
heads).  Data parallel over batch, tensor parallel over heads; the output
projection is column-sharded so each core produces a partial (1024, 2048)
output that the host sums per batch.

Fused single-pass pipeline.  The softmax EXP on the Scalar engine (~136us
total) is the phase-2 critical resource, so all projection work (context
quarters 2-3, Q for the second query block, out-projection of the first) is
injected as filler quanta between attention iterations to keep the PE busy
while ACT chews exp.  Per-head K^T/Q^T are duplicated into both partition
halves so the two query halves of a scores chunk run concurrently on array
tiles T0/T8 (row tiling, 2x scores throughput).

Host-side layouts (partition-major so DMA lines are large; all dram tensors
declared 2-D):
  xt/ctxt  [128, 8*2048]    embed e = p*8+k
  wqT/wkT/wvT [128, 8*256]  rows match the e = p*8+k mapping
  aqT/avT  [128, 8*128]
  bqT/bvT  [128, 256]       contraction = LoRA rank on partitions
  woT      [128, 2*1024]    row (p, j) = att layout dim j*128+p
  out_t    [1024, 2048]     f32, row e*128+p; host out[b] += out_t.T
Scale folding as baseline: 1/sqrt(hd) in wq/bq, 1/r in bq/bv.
"""

import numpy as np

import concourse.bass as bass
import concourse.tile as tile
from concourse import bacc, mybir
from concourse.bass import ts
from concourse.bass_utils import run_bass_kernel_spmd

F32 = mybir.dt.float32
BF16 = mybir.dt.bfloat16
EXP = mybir.ActivationFunctionType.Exp

# scores matmul mode: "pair" = chunk's two query halves on array tiles
# T0/T8 concurrently (2x scores throughput); "t0" = both halves on T0
# (64-contraction, no packing); "full" = 128-contraction over the duplicated
# halves (head counted twice, host folds an extra 1/2 into wk).
SCORES_MODE = "pair"

P = 128          # partitions
D = 1024         # embed dim
KO = D // P      # embed chunks (8)
HG = 4           # heads per core
HD = 64          # head dim
GD = HG * HD     # group dim (256)
R = 128          # LoRA rank
SQ = 1024        # query block (exp instruction free size)
QTR = 512        # ctx quarter


def build_nc(S=2048, C=2048):
    nc = bacc.Bacc("TRN2", target_bir_lowering=False, debug=False)

    xt = nc.dram_tensor("xt", [P, KO * S], BF16, kind="ExternalInput").ap()
    ctxt = nc.dram_tensor("ctxt", [P, KO * C], BF16, kind="ExternalInput").ap()
    wqT = nc.dram_tensor("wqT", [P, KO * GD], BF16, kind="ExternalInput").ap()
    aqT = nc.dram_tensor("aqT", [P, KO * R], BF16, kind="ExternalInput").ap()
    bqT = nc.dram_tensor("bqT", [R, GD], BF16, kind="ExternalInput").ap()
    wkT = nc.dram_tensor("wkT", [P, KO * GD], BF16, kind="ExternalInput").ap()
    wvT = nc.dram_tensor("wvT", [P, KO * GD], BF16, kind="ExternalInput").ap()
    avT = nc.dram_tensor("avT", [P, KO * R], BF16, kind="ExternalInput").ap()
    bvT = nc.dram_tensor("bvT", [R, GD], BF16, kind="ExternalInput").ap()
    woT = nc.dram_tensor("woT", [P, 2 * D], BF16, kind="ExternalInput").ap()
    out_t = nc.dram_tensor("out_t", [D, S], F32, kind="ExternalOutput").ap()

    xt = xt.rearrange("p (ko s) -> p ko s", ko=KO)
    ctxt = ctxt.rearrange("p (ko s) -> p ko s", ko=KO)
    wqT = wqT.rearrange("p (ko g) -> p ko g", ko=KO)
    aqT = aqT.rearrange("p (ko r) -> p ko r", ko=KO)
    wkT = wkT.rearrange("p (ko g) -> p ko g", ko=KO)
    wvT = wvT.rearrange("p (ko g) -> p ko g", ko=KO)
    avT = avT.rearrange("p (ko r) -> p ko r", ko=KO)
    woT = woT.rearrange("p (j d) -> p j d", j=2)
    out_t = out_t.rearrange("(ko p) s -> ko p s", p=P)
    with tile.TileContext(nc) as tc:
        _build(tc, xt, ctxt, wqT, aqT, bqT, wkT, wvT, avT, bvT, woT, out_t, S, C)
    nc.compile()
    return nc


def _build(tc, xt, ctxt, wqT, aqT, bqT, wkT, wvT, avT, bvT, woT, out_t, S, C):
    nc = tc.nc
    CK = C // P       # 16 ctx chunks
    NCQ = C // QTR    # 4 ctx quarters

    with (
        tc.tile_pool(name="w", bufs=1) as wpool,
        tc.tile_pool(name="acts", bufs=1) as actsp,
        tc.tile_pool(name="kqv", bufs=1) as kqv,
        tc.tile_pool(name="lora", bufs=1) as lorap,
        tc.tile_pool(name="pt", bufs=6) as ptp,
        tc.tile_pool(name="small", bufs=2) as smallp,
        tc.tile_pool(name="outsb", bufs=2) as outp,
        tc.tile_pool(name="work", bufs=3, space="PSUM") as work,
        tc.tile_pool(name="otp", bufs=1, space="PSUM") as otp,
    ):
        # ---- persistent SBUF tensors ----
        ctx_sb = actsp.tile([P, KO, C], BF16, tag="ctx")
        x_sb = actsp.tile([P, KO, S], BF16, tag="x")
        # per-head K^T / Q^T, duplicated into both partition halves so the
        # two query halves of a chunk can run on array tiles T0 and T8
        kt_sb = kqv.tile([P, HG, C], BF16, tag="kt")
        qt_sb = kqv.tile([P, HG, S], BF16, tag="qt")
        vaug_sb = kqv.tile([P, CK, HG, P], BF16, tag="vaug")
        att_sb = kqv.tile([P, 2, S], BF16, tag="att")   # normalized attn out^T
        tv_sb = lorap.tile([R, C], BF16, tag="tv")
        tq_sb = lorap.tile([R, S], BF16, tag="tq")

        aq_sb = wpool.tile([P, KO, R], BF16, tag="aq")
        av_sb = wpool.tile([P, KO, R], BF16, tag="av")
        bq_sb = wpool.tile([R, GD], BF16, tag="bq")
        bv_sb = wpool.tile([R, GD], BF16, tag="bv")
        wk_sb = wpool.tile([P, KO, GD], BF16, tag="wk")
        wv_sb = wpool.tile([P, KO, GD], BF16, tag="wv")
        wq_sb = wpool.tile([P, KO, GD], BF16, tag="wq")
        wo_sb = wpool.tile([P, 2, D], BF16, tag="wo")

        # ---- input DMAs, ordered by first use ----
        nc.sync.dma_start(ctx_sb[:, :, 0:QTR], ctxt[:, :, 0:QTR])
        nc.sync.dma_start(av_sb[:], avT)
        nc.sync.dma_start(wk_sb[:], wkT)
        nc.sync.dma_start(wv_sb[:], wvT)
        nc.sync.dma_start(bv_sb[:], bvT)
        nc.sync.dma_start(ctx_sb[:, :, QTR:2 * QTR], ctxt[:, :, QTR:2 * QTR])
        nc.sync.dma_start(x_sb[:, :, 0:SQ], xt[:, :, 0:SQ])
        nc.sync.dma_start(aq_sb[:], aqT)
        nc.sync.dma_start(wq_sb[:], wqT)
        nc.sync.dma_start(bq_sb[:], bqT)
        nc.sync.dma_start(ctx_sb[:, :, 2 * QTR:3 * QTR], ctxt[:, :, 2 * QTR:3 * QTR])
        nc.sync.dma_start(ctx_sb[:, :, 3 * QTR:4 * QTR], ctxt[:, :, 3 * QTR:4 * QTR])
        nc.sync.dma_start(x_sb[:, :, SQ:2 * SQ], xt[:, :, SQ:2 * SQ])
        nc.sync.dma_start(wo_sb[:], woT)

        # ones column for the attn-V rowsum; zero the pad columns so the
        # (unread) ot rows 65..127 accumulate zeros, not junk
        nc.vector.memset(vaug_sb[:, :, :, HD:P], 0.0)
        nc.vector.memset(vaug_sb[:, :, :, HD], 1.0)

        # ---- projection emitters (closures so they can be used as fillers) ----
        def quarter_closures(q):
            """Project ctx quarter q -> tv, Kt, Vaug.  4 filler quanta."""
            sl = ts(q, QTR)

            def tv():
                wt = work.tile([P, SQ], F32, tag="work")
                for k in range(KO):
                    nc.tensor.matmul(wt[:, 0:QTR], av_sb[:, k, :], ctx_sb[:, k, sl],
                                     start=(k == 0), stop=(k == KO - 1))
                nc.vector.tensor_copy(tv_sb[:, sl], wt[:, 0:QTR])

            def kproj(m):
                def f():
                    wt = work.tile([P, SQ], F32, tag="work")
                    for k in range(KO):
                        nc.tensor.matmul(wt[:, 0:QTR], wk_sb[:, k, ts(m, P)],
                                         ctx_sb[:, k, sl],
                                         start=(k == 0), stop=(k == KO - 1))
                    nc.vector.tensor_copy(kt_sb[0:HD, 2 * m, sl], wt[0:HD, 0:QTR])
                    nc.vector.tensor_copy(kt_sb[HD:P, 2 * m + 1, sl], wt[HD:P, 0:QTR])
                    nc.sync.dma_start(kt_sb[HD:P, 2 * m, sl], kt_sb[0:HD, 2 * m, sl])
                    nc.sync.dma_start(kt_sb[0:HD, 2 * m + 1, sl],
                                      kt_sb[HD:P, 2 * m + 1, sl])
                return f

            def vproj():
                wt = work.tile([P, SQ], F32, tag="work")
                vpq = wt.rearrange("p (m g) -> p m g", m=4)
                for mloc in range(4):
                    cpos = q * QTR + mloc * P
                    for k in range(KO):
                        nc.tensor.matmul(vpq[:, mloc, :], ctx_sb[:, k, cpos:cpos + P],
                                         wv_sb[:, k, :], start=(k == 0), stop=False)
                    nc.tensor.matmul(vpq[:, mloc, :], tv_sb[:, cpos:cpos + P],
                                     bv_sb[:], start=False, stop=True)
                nc.vector.tensor_copy(
                    vaug_sb[:, q * 4:(q + 1) * 4, :, 0:HD],
                    vpq.rearrange("p m (h d) -> p m h d", h=HG))

            return [tv, kproj(0), kproj(1), vproj]

        def q_closures(qb):
            """Project query block qb -> tq, Qt.  3 filler quanta."""
            sl = ts(qb, SQ)

            def tq():
                wt = work.tile([P, SQ], F32, tag="work")
                for n in range(2):
                    nsl = slice(qb * SQ + n * 512, qb * SQ + (n + 1) * 512)
                    for k in range(KO):
                        nc.tensor.matmul(wt[:, ts(n, 512)], aq_sb[:, k, :],
                                         x_sb[:, k, nsl],
                                         start=(k == 0), stop=(k == KO - 1))
                nc.vector.tensor_copy(tq_sb[:, sl], wt[:])

            def qproj(m):
                def f():
                    wt = work.tile([P, SQ], F32, tag="work")
                    for n in range(2):
                        nsl = slice(qb * SQ + n * 512, qb * SQ + (n + 1) * 512)
                        for k in range(KO):
                            nc.tensor.matmul(wt[:, ts(n, 512)],
                                             wq_sb[:, k, ts(m, P)], x_sb[:, k, nsl],
                                             start=(k == 0), stop=False)
                        nc.tensor.matmul(wt[:, ts(n, 512)], bq_sb[:, ts(m, P)],
                                         tq_sb[:, nsl], start=False, stop=True)
                    nc.vector.tensor_copy(qt_sb[0:HD, 2 * m, sl], wt[0:HD, :])
                    nc.vector.tensor_copy(qt_sb[HD:P, 2 * m + 1, sl], wt[HD:P, :])
                    nc.sync.dma_start(qt_sb[HD:P, 2 * m, sl], qt_sb[0:HD, 2 * m, sl])
                    nc.sync.dma_start(qt_sb[0:HD, 2 * m + 1, sl],
                                      qt_sb[HD:P, 2 * m + 1, sl])
                return f

            return [tq, qproj(0), qproj(1)]

        def outproj_closures(qb):
            """Out-projection of query block qb.  8 filler quanta (one per
            128-wide output chunk)."""
            sl = ts(qb, SQ)

            def op(e):
                def f():
                    wt = work.tile([P, SQ], F32, tag="work")
                    for n in range(2):
                        nsl = slice(qb * SQ + n * 512, qb * SQ + (n + 1) * 512)
                        for j in range(2):
                            nc.tensor.matmul(wt[:, ts(n, 512)],
                                             wo_sb[:, j, ts(e, P)], att_sb[:, j, nsl],
                                             start=(j == 0), stop=(j == 1))
                    ob = outp.tile([P, SQ], F32, tag="osb")
                    nc.vector.tensor_copy(ob[:], wt[:])
                    nc.sync.dma_start(out_t[e][:, sl], ob[:])
                return f

            return [op(e) for e in range(KO)]

        # ---- attention for one (query block, head) with filler injection ----
        def attention(qb, h, fillers, dens):
            hc, hp = h // 2, (h % 2) * HD
            qsl = ts(qb, SQ)
            ot = otp.tile([P, SQ], F32, tag="ot")
            pts = {}

            def attnv(i):
                pt = pts.pop(i)
                for n in range(2):
                    nc.tensor.matmul(ot[:, ts(n, 512)], vaug_sb[:, i, h, :],
                                     pt[:, ts(n, 512)],
                                     start=(i == 0), stop=(i == CK - 1))

            for i in range(CK):
                st = work.tile([P, SQ], F32, tag="work")
                nsl = lambda n: slice(qb * SQ + n * 512, qb * SQ + (n + 1) * 512)
                if SCORES_MODE == "full":
                    for n in range(2):
                        nc.tensor.matmul(st[:, ts(n, 512)], kt_sb[:, h, ts(i, P)],
                                         qt_sb[:, h, nsl(n)], start=True, stop=True)
                elif SCORES_MODE == "t0":
                    for n in range(2):
                        nc.tensor.matmul(st[:, ts(n, 512)], kt_sb[0:HD, h, ts(i, P)],
                                         qt_sb[0:HD, h, nsl(n)], start=True, stop=True)
                else:  # "pair": query half 0 on T0, half 1 on T8, concurrent
                    nc.tensor.matmul(st[:, 0:512], kt_sb[0:HD, h, ts(i, P)],
                                     qt_sb[0:HD, h, nsl(0)], start=True, stop=True)
                    nc.tensor.matmul(st[:, 512:1024], kt_sb[HD:P, h, ts(i, P)],
                                     qt_sb[HD:P, h, nsl(1)], start=True, stop=True)
                pt = ptp.tile([P, SQ], BF16, tag="pt")
                nc.scalar.activation(pt[:], st[:], EXP)
                pts[i] = pt
                if i >= 2:
                    attnv(i - 2)
                for _ in range(dens):
                    if fillers:
                        fillers.pop(0)()
            attnv(CK - 2)
            attnv(CK - 1)

            # normalize: rows 0..63 are O^T, row 64 is the exp rowsum.
            # Exact baseline chain: hop the rowsum through a partition-0
            # [1, N] tile before the reciprocal; multiply reads ot from PSUM.
            rr = smallp.tile([1, SQ], F32, tag="rr")
            nc.vector.tensor_copy(rr[:], ot[HD:HD + 1, :])
            onm = smallp.tile([HD, SQ], F32, tag="onm")
            nc.vector.tensor_copy(onm[:], ot[0:HD, :])
            rf = smallp.tile([1, SQ], F32, tag="rf")
            nc.vector.reciprocal_approx_fast(rf[:], rr[:])
            rb = smallp.tile([HD, SQ], F32, tag="rb")
            nc.gpsimd.partition_broadcast(rb[:], rf[:])
            nc.vector.tensor_mul(att_sb[hp:hp + HD, hc, qsl], onm[:], rb[:])

        # ================= emission =================
        # prologue: ctx quarters 0-1 and Q(qb0) inline
        for q in range(2):
            for c in quarter_closures(q):
                c()
        for c in q_closures(0):
            c()

        # qb0: fillers = ctx quarters 2-3 (needed by scores chunk >= 8 of h0,
        # emitted 2/iter so they land in the PE queue early), then Q(qb1).
        f0 = []
        for q in range(2, NCQ):
            f0 += quarter_closures(q)
        f0 += q_closures(1)
        attention(0, 0, f0, 2)
        attention(0, 1, f0, 1)
        attention(0, 2, f0, 1)
        attention(0, 3, f0, 1)
        assert not f0

        # qb1: fillers = out-projection of qb0
        f1 = outproj_closures(0)
        attention(1, 0, f1, 1)
        attention(1, 1, f1, 1)
        attention(1, 2, f1, 1)
        attention(1, 3, f1, 1)
        assert not f1

        # tail: out-projection of qb1
        for c in outproj_closures(1):
            c()


# ---------------------------------------------------------------------------
# Host side
# ---------------------------------------------------------------------------

_NC_CACHE = {}


def _get_nc(S=2048, C=2048):
    key = (S, C)
    if key not in _NC_CACHE:
        _NC_CACHE[key] = build_nc(S, C)
    return _NC_CACHE[key]


def shard_inputs(x, context, Wq, Aq, Bq, Wk, Wv, Av, Bv, Wo):
    """Build the 8 per-core input maps (host-side shard + transpose + scale +
    bf16 cast, partition-major 2-D layouts)."""
    import ml_dtypes

    bf16 = ml_dtypes.bfloat16
    f = lambda a: np.ascontiguousarray(np.asarray(a, dtype=np.float32))
    c = lambda a: np.ascontiguousarray(a).astype(bf16)
    x, context = f(x), f(context)
    Wq, Aq, Bq, Wk, Wv, Av, Bv, Wo = map(f, (Wq, Aq, Bq, Wk, Wv, Av, Bv, Wo))
    sd = 8.0   # sqrt(head_dim)
    lr = 128.0  # LoRA rank (scale = 1/r)
    aqT = c(Aq.T.reshape(P, KO * R))
    avT = c(Av.T.reshape(P, KO * R))
    in_maps = []
    for core in range(8):
        b, g = core // 4, core % 4
        sl = slice(g * GD, (g + 1) * GD)
        in_maps.append({
            "xt": c(x[b].T.reshape(P, KO * 2048)),
            "ctxt": c(context[b].T.reshape(P, KO * 2048)),
            "wqT": c((Wq[sl].T / sd).reshape(P, KO * GD)),
            "aqT": aqT,
            "bqT": c(Bq[sl].T / (lr * sd)),
            "wkT": c((Wk[sl] / (2.0 if SCORES_MODE == "full" else 1.0))
                     .T.reshape(P, KO * GD)),
            "wvT": c(Wv[sl].T.reshape(P, KO * GD)),
            "avT": avT,
            "bvT": c(Bv[sl].T / lr),
            "woT": c(np.ascontiguousarray(
                Wo[:, sl].T.reshape(2, P, D).swapaxes(0, 1)).reshape(P, 2 * D)),
        })
    return in_maps


def unshard_output(results, B=2, S=2048):
    out = np.zeros((B, S, D), np.float32)
    for core in range(8):
        b = core // 4
        out[b] += results[core]["out_t"].reshape(D, S).astype(np.float32).T
    return out


def kernel(x, context, Wq, Aq, Bq, Wk, Wv, Av, Bv, Wo, _trace=False):
    nc = _get_nc()
    in_maps = shard_inputs(x, context, Wq, Aq, Bq, Wk, Wv, Av, Bv, Wo)
    res = run_bass_kernel_spmd(nc, in_maps, core_ids=list(range(8)), trace=_trace)
    out = unshard_output(res.results)
    if _trace:
        kernel.last_result = res
    return out
